# revision 64
# baseline (speedup 1.0000x reference)
"""Trainium2 Bass kernel for nn_FLASH_40458591928592 (sparse_attention).

Sequence-sharded over 8 NeuronCores: 1024 tokens (= 4 groups of 256) per core.
Matmuls bf16 operands / fp32 PSUM, except:
  - v GEMM (x0 @ Wv): fp8e4m3 DoubleRow (2x PE rate), Wv host-scaled x64,
    silu applied with scale 1/64.
  - out GEMM (og @ W_out): first N8PAIRS*2 of 16 k-tiles in fp8 DoubleRow
    (og scaled x8 on chip, W_out x64 on host), rest bf16 with matching x512
    scale so all terms share one PSUM chain; evacuation scales by 1/512.
Residual x is added from bf16 (error contribution ~2e-3 rel).

Per-core device program:
  qk   = silu(x_s @ W_qk + b_qk)            feature-major  [128qk, 1024tok]
  v    = silu(x_0 @ W_h[:, :H] + b_h[:H])   token-major    [1024tok, 2048]
  gate = silu(x_s @ W_h[:, H:] + b_h[H:])   feature-major  [2048hc, 1024tok]
  Quadratic group attention with the causal depthwise conv folded in as a
  constant banded add to attnT plus a K=32 boundary matmul over the 32-token
  tail of the previous group (halo-computed for the first local group).
  lin_kv prefix scan: per-group kv accumulated in one PSUM bank with
  snapshots; per-core totals AllGathered per e-half (bf16); each core applies
  a 0/1 weighted sum (host-provided weights) for the exclusive prefix.
  out  = quadT + linT + convT               feature-major  [2048, 1024tok]
  y_s  = (gate_s*out)^T @ W_out + b_out + x_s  token-major, written fp32.

The first gate block (s0/th0) is emitted with deferred og-muls between
lin_half(0) and the eh1 collective consume, so the PE has ~27us of work
covering the second AllGather's latency.
"""

from contextlib import ExitStack

import numpy as np
import ml_dtypes

import concourse.tile as tile
from concourse import bacc, mybir
from concourse.bass_utils import run_bass_kernel_spmd
from concourse.masks import make_identity

BF = mybir.dt.bfloat16
F8 = mybir.dt.float8e4
F32 = mybir.dt.float32
bf16 = ml_dtypes.bfloat16
f8e4 = ml_dtypes.float8_e4m3

G = 256
DIM = 1024
HID = 2048
DQK = 128
NSEQ = 8192
NC = 8
T = NSEQ // NC        # 1024 tokens per core
NG = T // G           # 4 groups per core
KD = DIM // 128       # 8 k-tiles over dim
ET = HID // 128       # 16 e-tiles over hid
TT = T // 128         # 8 token tiles
EH = HID // 2         # 1024 cols per e-half

AF = mybir.ActivationFunctionType
ALU = mybir.AluOpType
DR = mybir.MatmulPerfMode.DoubleRow

WITH_VBIAS = True   # emit ones-row matmuls for b_h[:HID] (skipped when zero)
WITH_OBIAS = True   # emit ones-row matmuls for b_out (skipped when zero)

N8PAIRS = 6          # out-proj k-tile pairs in fp8 (of ET//2=8); rest bf16
S_W = 64.0           # host pre-scale on fp8 weights
S_OG = 8.0           # on-chip pre-scale on fp8/bf16 og
S_OUT = S_W * S_OG   # combined out-proj psum scale


def _emit(tc, ap):
    nc = tc.nc
    nf8 = N8PAIRS * 2          # fp8 k-tiles in out-proj
    with ExitStack() as ctx:
        consts = ctx.enter_context(tc.tile_pool(name="consts", bufs=1))
        p_xt = ctx.enter_context(tc.tile_pool(name="xt", bufs=3))
        p_xf8 = ctx.enter_context(tc.tile_pool(name="xf8", bufs=1))
        p_qkT = ctx.enter_context(tc.tile_pool(name="qkT", bufs=1))
        p_lk = ctx.enter_context(tc.tile_pool(name="lk", bufs=1))
        p_w = ctx.enter_context(tc.tile_pool(name="w", bufs=1))      # Wv -> Wg
        p_sw = ctx.enter_context(tc.tile_pool(name="sw", bufs=1))    # S_full -> Woutb
        p_w8 = ctx.enter_context(tc.tile_pool(name="w8", bufs=1))    # Wout8
        p_big = ctx.enter_context(tc.tile_pool(name="big", bufs=1))  # v_h -> gsilu
        p_og8 = ctx.enter_context(tc.tile_pool(name="og8", bufs=2))
        p_tails = ctx.enter_context(tc.tile_pool(name="tails", bufs=1))
        p_so = ctx.enter_context(tc.tile_pool(name="so", bufs=1))
        p_sob = ctx.enter_context(tc.tile_pool(name="sob", bufs=1))
        p_tr = ctx.enter_context(tc.tile_pool(name="tr", bufs=2))
        p_a0 = ctx.enter_context(tc.tile_pool(name="a0", bufs=4))
        p_a1 = ctx.enter_context(tc.tile_pool(name="a1", bufs=4))
        p_outT = ctx.enter_context(tc.tile_pool(name="outT", bufs=1))
        p_y = ctx.enter_context(tc.tile_pool(name="ysb", bufs=2))
        p_xr = ctx.enter_context(tc.tile_pool(name="xr", bufs=2))
        ps1 = ctx.enter_context(tc.tile_pool(name="ps1", bufs=4, space="PSUM"))
        ps2 = ctx.enter_context(tc.tile_pool(name="ps2", bufs=2, space="PSUM"))
        pskv = ctx.enter_context(tc.tile_pool(name="pskv", bufs=2, space="PSUM"))

        # warm-up collective FIRST: its ~50us setup barrier must finish
        # before the real AllGathers reach the cc stream (~70us in). The
        # payload is never read — fire on the uninitialized buffer so the
        # barrier starts immediately
        nc.gpsimd.collective_compute(
            "AllGather", ALU.bypass, replica_groups=[list(range(NC))],
            ins=[ap["cc_warm_in"]], outs=[ap["cc_warm_out"]])

        # ---- the DMAs that gate the first PE work ----
        # consts/weights go on the ACT HWDGE queue so xt streams don't queue
        # behind them on SP
        wqk_sb = consts.tile([128, KD, DQK], BF, tag="wqk")
        nc.scalar.dma_start(wqk_sb, ap["wqk"].rearrange("(kt p) q -> p kt q", p=128))

        def load_xt(s, scalar_late=False):
            halves = []
            for h in range(2):
                t = p_xt.tile([128, KD // 2, T], BF, tag="xt")
                # one DMA per k-tile, halves split across the SP and GpSimd
                # HWDGE queues — one queue's engine fan-out (~100GB/s) is
                # less than the qk streams consume. GpSimd's h1 halves drain
                # late behind the warmup collective, but the extra aggregate
                # bandwidth still wins (measured: 624us vs 643us sync-only).
                # scalar_late: both halves on ACT, which idles once the
                # weights drain (~32us) — used for the last qk stream
                if scalar_late:
                    eng = nc.scalar
                else:
                    eng = nc.sync if h == 0 else nc.gpsimd
                for q in range(4):
                    r0 = h * 512 + q * 128
                    eng.dma_start(
                        t[:, q:q + 1, :],
                        ap["xt"][s, r0:r0 + 128, :].rearrange(
                            "(kt p) t -> p kt t", p=128))
                halves.append(t)
            return halves

        def xt_k(halves, kt):
            return halves[kt // 4][:, kt % 4, :]

        bqk = consts.tile([128, 1], F32, tag="bqk")
        nc.scalar.dma_start(bqk, ap["bqk"])

        qkT = p_qkT.tile([128, 4, T], BF, tag="qkT")

        # prefetch Wv (fp8) early (first big PE phase after qk needs it)
        wv_sb = p_w.tile([128, KD, HID], F8, tag="wv8")
        nc.scalar.dma_start(wv_sb, ap["wv"].rearrange("(kt p) e -> p kt e", p=128))
        # x0 in fp8 for the v GEMM
        xf8 = p_xf8.tile([128, KD, T], F8, tag="xf8")
        nc.scalar.dma_start(xf8, ap["x0f8"].rearrange("(kt p) t -> p kt t", p=128))
        wout8_sb = p_w8.tile([128, max(nf8, 1), DIM], F8, tag="w8")

        def qk_stream(s, halves):
            # half-major: the stream starts once its first xt half lands;
            # ch0/ch1 share each LDWEIGHTS
            psa = ps1.tile([128, 512], F32, tag="ps1")
            psb = ps1.tile([128, 512], F32, tag="ps1")
            pss = [psa, psb]
            for h in range(2):
                for q in range(4):
                    kt = 4 * h + q
                    for ch in range(2):
                        nc.tensor.matmul(
                            pss[ch], wqk_sb[:, kt, :],
                            halves[h][:, q, ch * 512:(ch + 1) * 512],
                            start=(kt == 0), stop=(kt == KD - 1))
            for ch in range(2):
                nc.scalar.activation(qkT[:, s, ch * 512:(ch + 1) * 512], pss[ch],
                                     AF.Silu, bias=bqk, scale=1.0)

        halves3 = load_xt(3)

        # ---- constants (DMA on ACT queue while xt s3 streams) ----
        ident = consts.tile([128, 128], BF, tag="ident")
        make_identity(nc, ident)
        ones_t = None
        if WITH_VBIAS or WITH_OBIAS:
            ones_t = consts.tile([1, 128], BF, tag="ones")
            nc.vector.memset(ones_t, 1.0)
        triu = consts.tile([128, 128], BF, tag="triu")
        nc.scalar.dma_start(triu, ap["triu"])
        bdiag = consts.tile([128, 128], BF, tag="bdiag")
        nc.scalar.dma_start(bdiag, ap["bdiag"])
        bcorn = consts.tile([128, 128], BF, tag="bcorn")
        nc.scalar.dma_start(bcorn, ap["bcorn"])
        bprev = consts.tile([32, 32], BF, tag="bprev")
        nc.scalar.dma_start(bprev, ap["bprev"])
        hmask = consts.tile([32, 1], F32, tag="hmask")
        nc.scalar.dma_start(hmask, ap["hmask"])
        wsumw = consts.tile([128, NC], F32, tag="wsumw")
        nc.scalar.dma_start(wsumw, ap["wsumw"])
        bgate = consts.tile([128, ET], F32, tag="bgate")
        nc.scalar.dma_start(bgate, ap["bgate"])
        wvb = bout = None
        if WITH_VBIAS:
            wvb = consts.tile([1, HID], BF, tag="wvb")
            nc.scalar.dma_start(wvb, ap["wvb"])
        if WITH_OBIAS:
            bout = consts.tile([1, DIM], BF, tag="bout")
            nc.scalar.dma_start(bout, ap["bout"])
        xh_sb = consts.tile([128, KD, 32], F8, tag="xh")
        nc.scalar.dma_start(xh_sb, ap["xh"].rearrange("(kt p) t -> p kt t", p=128))

        qk_stream(3, halves3)

        # lk (stream 3) token-major via PE transpose
        lk_tok = p_lk.tile([128, TT, 128], BF, tag="lk")
        for tt in range(TT):
            pt = ps2.tile([128, 128], BF, tag="ps2")
            nc.tensor.transpose(pt, qkT[:, 3, tt * 128:(tt + 1) * 128], ident)
            nc.vector.tensor_copy(lk_tok[:, tt, :], pt)

        # ---- sim/attn per group (emitted after qk streams 0/2 are in) ----
        attn0, attn1 = [], []

        def attn_build():
            for g in range(NG):
                i0 = g * G
                a0 = p_a0.tile([128, 256], BF, tag="a0")
                ps = ps2.tile([128, 256], F32, tag="ps2")
                nc.tensor.matmul(ps, qkT[:, 2, i0:i0 + 128],
                                 qkT[:, 0, i0:i0 + 256], start=True, stop=True)
                nc.scalar.activation(a0, ps, AF.Relu, bias=0.0, scale=1.0 / G)
                nc.vector.tensor_mul(a0[:, 0:128], a0[:, 0:128], triu)
                nc.vector.tensor_mul(a0, a0, a0)
                nc.vector.tensor_add(a0[:, 0:128], a0[:, 0:128], bdiag)
                nc.vector.tensor_add(a0[:, 128:256], a0[:, 128:256], bcorn)
                attn0.append(a0)

                a1 = p_a1.tile([128, 128], BF, tag="a1")
                ps = ps2.tile([128, 256], F32, tag="ps2")
                nc.tensor.matmul(ps[:, 0:128], qkT[:, 2, i0 + 128:i0 + 256],
                                 qkT[:, 0, i0 + 128:i0 + 256],
                                 start=True, stop=True)
                nc.scalar.activation(a1, ps[:, 0:128], AF.Relu, bias=0.0,
                                     scale=1.0 / G)
                nc.vector.tensor_mul(a1, a1, triu)
                nc.vector.tensor_mul(a1, a1, a1)
                nc.vector.tensor_add(a1, a1, bdiag)
                attn1.append(a1)

        # ---- phases B-E per e-half ----
        S_full = p_sw.tile([128, NG, HID], BF, tag="sw")
        S_offb = p_sob.tile([128, HID], BF, tag="sob")

        def fire_half(eh, t_half):
            """DMA totals out and launch this half's AllGather (small payloads
            overlap compute far better than one combined collective)."""
            cc_in = ap[f"cc_in{eh}"]
            nc.sync.dma_start(cc_in, t_half)
            nc.gpsimd.collective_compute(
                "AllGather", ALU.bypass, replica_groups=[list(range(NC))],
                ins=[cc_in], outs=[ap[f"cc_out{eh}"]])

        def consume_half(eh):
            """Weighted-sum the gathered totals, add into S_full (DVE)."""
            e0 = eh * EH
            cc_out = ap[f"cc_out{eh}"]
            for r in range(NC):
                tr = p_tr.tile([128, EH], BF, tag="tr")
                nc.sync.dma_start(tr, cc_out[r * 128:(r + 1) * 128, :])
                if r == 0:
                    nc.vector.tensor_scalar_mul(S_offb[:, e0:e0 + EH], tr,
                                                wsumw[:, 0:1])
                else:
                    nc.vector.scalar_tensor_tensor(
                        S_offb[:, e0:e0 + EH], tr, wsumw[:, r:r + 1],
                        S_offb[:, e0:e0 + EH], op0=ALU.mult, op1=ALU.add)
            nc.vector.tensor_copy(S_full[:, 0, e0:e0 + EH], S_offb[:, e0:e0 + EH])
            for g in range(1, NG):
                nc.vector.tensor_add(S_full[:, g, e0:e0 + EH],
                                     S_full[:, g, e0:e0 + EH],
                                     S_offb[:, e0:e0 + EH])

        outT = p_outT.tile([128, ET, T], BF, tag="outT")

        def lin_half(eh):
            for g in range(NG):
                for et in range(eh * 8, eh * 8 + 8):
                    po = ps2.tile([128, 256], F32, tag="ps2")
                    nc.tensor.matmul(po, S_full[:, g, et * 128:(et + 1) * 128],
                                     qkT[:, 1, g * G:(g + 1) * G],
                                     start=True, stop=True)
                    nc.vector.tensor_add(outT[:, et, g * G:(g + 1) * G],
                                         outT[:, et, g * G:(g + 1) * G], po)

        def v_mm(ps, lhs_ap, e0, c0, w):
            """fp8 DoubleRow x@Wv chain into psum (scale S_W); w = token width."""
            for q in range(KD // 2):
                nc.tensor.matmul(
                    ps, lhs_ap[:, 2 * q:2 * q + 2, :],
                    wv_sb[:, 2 * q:2 * q + 2, e0 + c0:e0 + c0 + 512],
                    start=(q == 0),
                    stop=(q == KD // 2 - 1 and not WITH_VBIAS),
                    perf_mode=DR, skip_group_check=True)
            if WITH_VBIAS:
                nc.tensor.matmul(ps, ones_t[0:1, 0:w],
                                 wvb[0:1, e0 + c0:e0 + c0 + 512],
                                 start=False, stop=True, skip_group_check=True)

        def v_head(eh):
            v_h = p_big.tile([128, TT, EH], BF, tag="big")
            tails = p_tails.tile([32, NG, EH], BF, tag="tails")
            t_half = p_so.tile([128, EH], BF, tag="so")
            pk0 = pskv.tile([128, 512], F32, tag="pskv")
            pk1 = pskv.tile([128, 512], F32, tag="pskv")
            return dict(e0=eh * EH, v_h=v_h, tails=tails, t_half=t_half,
                        pk=[pk0, pk1])

        def v_pair(st, g):
            """Two v token-tiles + their kv-chain step (PSUM snapshot)."""
            e0, v_h, pk = st["e0"], st["v_h"], st["pk"]
            for tt in (2 * g, 2 * g + 1):
                for ec in range(2):
                    c0 = ec * 512
                    ps = ps1.tile([128, 512], F32, tag="ps1")
                    v_mm(ps, xf8[:, :, tt * 128:(tt + 1) * 128], e0, c0, 128)
                    nc.scalar.activation(v_h[:, tt, c0:c0 + 512], ps, AF.Silu,
                                         bias=0.0, scale=1.0 / S_W)
            for ec in range(2):
                c0 = ec * 512
                for jt in range(2):
                    nc.tensor.matmul(pk[ec], lk_tok[:, 2 * g + jt, :],
                                     v_h[:, 2 * g + jt, c0:c0 + 512],
                                     start=(g == 0 and jt == 0),
                                     stop=(g == NG - 1 and jt == 1),
                                     skip_group_check=True)
                dst = (S_full[:, g + 1, e0 + c0:e0 + c0 + 512]
                       if g < NG - 1 else st["t_half"][:, c0:c0 + 512])
                nc.scalar.activation(dst, pk[ec], AF.Copy, bias=0.0,
                                     scale=1.0 / G)

        def v_tail(eh, st):
            """Halo v for the conv boundary, group tails, fire the AllGather."""
            e0, v_h, tails = st["e0"], st["v_h"], st["tails"]
            for ec in range(2):
                c0 = ec * 512
                ps = ps1.tile([32, 512], F32, tag="ps1")
                v_mm(ps, xh_sb, e0, c0, 32)
                nc.scalar.activation(tails[:, 0, c0:c0 + 512], ps, AF.Silu,
                                     bias=0.0, scale=1.0 / S_W)
                nc.vector.tensor_scalar_mul(tails[:, 0, c0:c0 + 512],
                                            tails[:, 0, c0:c0 + 512], hmask)
            for g in range(1, NG):
                nc.sync.dma_start(tails[:, g, :], v_h[96:128, 2 * g - 1, :])
            fire_half(eh, st["t_half"])

        def quad_half(eh, st, g_order):
            v_h, tails = st["v_h"], st["tails"]
            for g in g_order:
                for et in range(8):
                    ec0 = et * 128
                    po = ps2.tile([128, 256], F32, tag="ps2")
                    nc.tensor.matmul(po, v_h[:, 2 * g, ec0:ec0 + 128], attn0[g],
                                     start=True, stop=False, skip_group_check=True)
                    nc.tensor.matmul(po[:, 128:256],
                                     v_h[:, 2 * g + 1, ec0:ec0 + 128],
                                     attn1[g], start=False, stop=False,
                                     skip_group_check=True)
                    nc.tensor.matmul(po[:, 0:32], tails[:, g, ec0:ec0 + 128], bprev,
                                     start=False, stop=True, skip_group_check=True)
                    nc.scalar.activation(outT[:, eh * 8 + et, g * G:(g + 1) * G],
                                         po, AF.Copy, bias=0.0, scale=1.0)

        # ---- interleaved: eh0 v-phase fills the qk streams' DMA windows ----
        st0 = v_head(0)
        v_pair(st0, 0)
        halves = load_xt(1)
        qk_stream(1, halves)
        v_pair(st0, 1)
        halves = load_xt(2)
        qk_stream(2, halves)
        v_pair(st0, 2)
        xt0 = load_xt(0, scalar_late=True)
        qk_stream(0, xt0)
        v_pair(st0, 3)
        # Wout fp8 part: first needed ~160us; on GpSimd AFTER the xt h1
        # loads so its 1.5MB stays out of the DMA-starved qk window
        if nf8:
            nc.gpsimd.dma_start(wout8_sb[:, 0:nf8, :],
                                ap["wout8"].rearrange("(kt p) n -> p kt n", p=128))
        v_tail(0, st0)          # fires AG0

        attn_build()
        consume_half(0)         # DVE parks on AG0 under quad0/eh1-v PE work
        quad_half(0, st0, range(NG))

        # ---- eh1 v-phase ----
        st1 = v_head(1)
        for g in range(NG):
            v_pair(st1, g)
        v_tail(1, st1)          # fires AG1; last Wv reader = the halo above

        # Wg chunked per-et so the gate can consume as it streams (the p_w
        # slot only frees after the eh1 halo); chunks alternate across two
        # queues — one queue's ~45GB/s starves the gate's 1.7us/et pace
        wg_sb = p_w.tile([128, KD, HID], BF, tag="w")
        for et in range(ET):
            eng = nc.scalar if et % 2 == 0 else nc.gpsimd
            eng.dma_start(
                wg_sb[:, :, et * 128:(et + 1) * 128],
                ap["wg"][:, et * 128:(et + 1) * 128].rearrange(
                    "(kt p) e -> p kt e", p=128))

        # Woutb (bf16 k-tiles) reuses the xf8 slot, dead after the eh1 halo;
        # loading here (not after lin1) keeps y_th0's bf16 chains fed
        woutb_sb = p_xf8.tile([128, max(ET - nf8, 1), DIM], BF, tag="xf8")
        for j in range(ET - nf8):
            nc.scalar.dma_start(woutb_sb[:, j, :],
                                ap["woutb"][j * 128:(j + 1) * 128, :])

        tc.no_sync_barrier()

        def gate_th(s, halves, th, defer_muls=False):
            t0 = th * 512
            gs = p_big.tile([128, ET, 512], BF, tag="big")
            for et in range(ET):
                ps = ps1.tile([128, 512], F32, tag="ps1")
                for kt in range(KD):
                    nc.tensor.matmul(
                        ps, wg_sb[:, kt, et * 128:(et + 1) * 128],
                        xt_k(halves, kt)[:, t0:t0 + 512],
                        start=(kt == 0), stop=(kt == KD - 1))
                nc.scalar.activation(gs[:, et, :], ps, AF.Silu,
                                     bias=bgate[:, et:et + 1], scale=1.0)
            if defer_muls:
                return gs, None
            return gs, og_muls(gs, th)

        def og_muls(gs, th, lo=0, hi=ET, og8=None):
            """og = S_OG * gate * outT; first nf8 k-tiles to fp8, rest bf16
            in place (so all out-proj terms carry the same x512 scale)."""
            t0 = th * 512
            if og8 is None and nf8:
                og8 = p_og8.tile([128, nf8, 512], F8, tag="og8")
            for et in range(lo, hi):
                if et < nf8:
                    nc.vector.scalar_tensor_tensor(
                        og8[:, et, :], gs[:, et, :], S_OG,
                        outT[:, et, t0:t0 + 512], op0=ALU.mult, op1=ALU.mult)
                else:
                    nc.vector.scalar_tensor_tensor(
                        gs[:, et, :], gs[:, et, :], S_OG,
                        outT[:, et, t0:t0 + 512], op0=ALU.mult, op1=ALU.mult)
            return og8

        def y_th(s, gs, og8, th):
            for tl in range(4):
                tt = th * 4 + tl
                for nch in range(2):
                    n0 = nch * 512
                    ps = ps1.tile([128, 512], F32, tag="ps1")
                    for p in range(N8PAIRS):
                        nc.tensor.matmul(
                            ps, og8[:, 2 * p:2 * p + 2, tl * 128:(tl + 1) * 128],
                            wout8_sb[:, 2 * p:2 * p + 2, n0:n0 + 512],
                            start=(p == 0),
                            stop=(nf8 == ET and p == N8PAIRS - 1
                                  and not WITH_OBIAS),
                            perf_mode=DR, skip_group_check=True)
                    for j, kt in enumerate(range(nf8, ET)):
                        nc.tensor.matmul(
                            ps, gs[:, kt, tl * 128:(tl + 1) * 128],
                            woutb_sb[:, j, n0:n0 + 512],
                            start=(N8PAIRS == 0 and j == 0),
                            stop=(kt == ET - 1 and not WITH_OBIAS),
                            skip_group_check=True)
                    if WITH_OBIAS:
                        nc.tensor.matmul(ps, ones_t[0:1, 0:128],
                                         bout[0:1, n0:n0 + 512],
                                         start=False, stop=True,
                                         skip_group_check=True)
                    xr = p_xr.tile([128, 512], BF, tag="xr")
                    nc.sync.dma_start(
                        xr, ap["xtok"][s, tt * 128:(tt + 1) * 128, n0:n0 + 512])
                    ysb = p_y.tile([128, 512], BF, tag="ysb")
                    nc.vector.scalar_tensor_tensor(
                        ysb, ps, 1.0 / S_OUT, xr, op0=ALU.mult, op1=ALU.add)
                    nc.sync.dma_start(
                        ap["y"][s, tt * 128:(tt + 1) * 128, n0:n0 + 512], ysb)

        # lin eh0 can run now (its collective was consumed during eh1's v);
        # then the deferred first gate block keeps the PE busy while the eh1
        # AllGather lands, after which its consume + lin eh1 complete outT.
        # quad1, lin0, then the deferred gate block cover AG1's latency on
        # the PE; quad1 must precede the gate block (its reads release the
        # p_big slot the gate's gs tile reuses). Group 0 last: its boundary
        # tails wait on the eh1 halo DVE mask.
        quad_half(1, st1, [1, 2, 3, 0])
        lin_half(0)
        gs0, _ = gate_th(0, xt0, 0, defer_muls=True)
        tc.no_sync_barrier()
        consume_half(1)
        lin_half(1)
        og8_0 = og_muls(gs0, 0)
        y_th(0, gs0, og8_0, 0)

        # ---- remaining gate + y blocks ----
        for s in range(4):
            halves = xt0 if s == 0 else load_xt(s)
            for th in range(2):
                if s == 0 and th == 0:
                    continue
                gs, og8 = gate_th(s, halves, th)
                y_th(s, gs, og8, th)


def build_nc(with_vbias=None, with_obias=None):
    global WITH_VBIAS, WITH_OBIAS
    if with_vbias is not None:
        WITH_VBIAS = with_vbias
    if with_obias is not None:
        WITH_OBIAS = with_obias
    nc = bacc.Bacc("TRN2", target_bir_lowering=False, debug=False, num_devices=NC)
    ap = {}
    nf8 = N8PAIRS * 2

    def dram(name, shape, dt, kind=None, addr_space=None):
        kw = {}
        if kind:
            kw["kind"] = kind
        if addr_space:
            kw["addr_space"] = addr_space
        ap[name] = nc.dram_tensor(name, shape, dt, **kw).ap()

    dram("xt", [4, DIM, T], BF, kind="ExternalInput")
    dram("x0f8", [DIM, T], F8, kind="ExternalInput")
    dram("xh", [DIM, 32], F8, kind="ExternalInput")
    dram("xtok", [4, T, DIM], BF, kind="ExternalInput")
    dram("wv", [DIM, HID], F8, kind="ExternalInput")
    dram("wg", [DIM, HID], BF, kind="ExternalInput")
    dram("wqk", [DIM, DQK], BF, kind="ExternalInput")
    dram("wout8", [max(nf8, 1) * 128, DIM], F8, kind="ExternalInput")
    dram("woutb", [max(ET - nf8, 1) * 128, DIM], BF, kind="ExternalInput")
    dram("wvb", [1, HID], BF, kind="ExternalInput")
    dram("bout", [1, DIM], BF, kind="ExternalInput")
    dram("bgate", [128, ET], F32, kind="ExternalInput")
    dram("bqk", [128, 1], F32, kind="ExternalInput")
    dram("triu", [128, 128], BF, kind="ExternalInput")
    dram("bdiag", [128, 128], BF, kind="ExternalInput")
    dram("bcorn", [128, 128], BF, kind="ExternalInput")
    dram("bprev", [32, 32], BF, kind="ExternalInput")
    dram("hmask", [32, 1], F32, kind="ExternalInput")
    dram("wsumw", [128, NC], F32, kind="ExternalInput")
    dram("cc_warm_in", [128, 16], BF)
    dram("cc_warm_out", [NC * 128, 16], BF, addr_space="Shared")
    dram("cc_in0", [128, EH], BF)
    dram("cc_out0", [NC * 128, EH], BF, addr_space="Shared")
    dram("cc_in1", [128, EH], BF)
    dram("cc_out1", [NC * 128, EH], BF, addr_space="Shared")
    dram("y", [4, T, DIM], BF, kind="ExternalOutput")

    with tile.TileContext(nc) as tc:
        _emit(tc, ap)
    nc.compile()
    return nc


def host_prep(inputs):
    """Pure layout transforms: shard, transpose, cast, build conv-band consts."""
    x = np.ascontiguousarray(np.asarray(inputs["x"], np.float32)[0])  # [4, N, DIM]
    W_h = np.asarray(inputs["W_h"], np.float32)
    b_h = np.asarray(inputs["b_h"], np.float32)
    W_qk = np.asarray(inputs["W_qk"], np.float32)
    b_qk = np.asarray(inputs["b_qk"], np.float32)
    W_out = np.asarray(inputs["W_out"], np.float32)
    b_out = np.asarray(inputs["b_out"], np.float32)
    cw = np.asarray(inputs["conv_w"], np.float32)
    nf8 = N8PAIRS * 2

    jj = np.arange(128)[:, None]
    ii = np.arange(128)[None, :]
    d = ii - jj
    triu = (ii >= jj).astype(bf16)
    bdiag = np.where((d >= 0) & (d <= 31), cw[np.clip(31 - d, 0, 62)], 0.0).astype(bf16)
    dc = (ii + 128) - jj
    bcorn = np.where((dc >= 0) & (dc <= 31),
                     cw[np.clip(31 - dc, 0, 62)], 0.0).astype(bf16)
    jt = np.arange(32)[:, None]
    ip = np.arange(32)[None, :]
    dp = ip + 32 - jt
    bprev = np.where((dp >= 1) & (dp <= 31),
                     cw[np.clip(31 - dp, 0, 62)], 0.0).astype(bf16)

    wout_s = W_out * S_W
    wout8 = wout_s[:nf8 * 128].astype(f8e4)
    if nf8 == 0:
        wout8 = np.zeros((128, DIM), f8e4)
    woutb = np.ascontiguousarray(wout_s[nf8 * 128:]).astype(bf16)
    if nf8 == ET:
        woutb = np.zeros((128, DIM), bf16)
    common = {
        "wv": (np.ascontiguousarray(W_h[:, :HID]) * S_W).astype(f8e4),
        "wg": np.ascontiguousarray(W_h[:, HID:]).astype(bf16),
        "wqk": W_qk.astype(bf16),
        "wout8": wout8,
        "woutb": woutb,
        "wvb": (b_h[None, :HID] * S_W).astype(bf16),
        "bout": (b_out[None, :] * S_OUT).astype(bf16),
        "bgate": np.ascontiguousarray(b_h[HID:].reshape(ET, 128).T).astype(np.float32),
        "bqk": b_qk[:, None].astype(np.float32),
        "triu": triu, "bdiag": bdiag, "bcorn": bcorn, "bprev": bprev,
    }

    in_maps = []
    for c in range(NC):
        sl = slice(c * T, (c + 1) * T)
        x_c = x[:, sl, :]
        xt = np.zeros((4, DIM, T), bf16)
        for s in range(4):
            xt[s] = x_c[s].T.astype(bf16)
        if c > 0:
            xh = np.ascontiguousarray(x[0, c * T - 32:c * T, :].T).astype(f8e4)
        else:
            xh = np.zeros((DIM, 32), f8e4)
        m = dict(common)
        m["xt"] = xt
        m["x0f8"] = np.ascontiguousarray(x_c[0].T).astype(f8e4)
        m["xh"] = xh
        m["xtok"] = np.ascontiguousarray(x_c).astype(bf16)
        m["hmask"] = np.full((32, 1), 1.0 if c > 0 else 0.0, np.float32)
        w = np.zeros((128, NC), np.float32)
        w[:, :c] = 1.0
        m["wsumw"] = w
        in_maps.append(m)
    return in_maps


_NC_PROG = None
_NC_FLAGS = None


def kernel(**inputs):
    global _NC_PROG, _NC_FLAGS
    b_h = np.asarray(inputs["b_h"], np.float32)
    b_out = np.asarray(inputs["b_out"], np.float32)
    flags = (bool(np.any(b_h[:HID])), bool(np.any(b_out)))
    if _NC_PROG is None or _NC_FLAGS != flags:
        _NC_PROG = build_nc(with_vbias=flags[0], with_obias=flags[1])
        _NC_FLAGS = flags
    in_maps = host_prep(inputs)
    res = run_bass_kernel_spmd(_NC_PROG, in_maps, list(range(NC)))
    y = np.stack([res.results[c]["y"] for c in range(NC)], axis=1)  # [4, NC, T, DIM]
    return np.ascontiguousarray(y.reshape(4, NSEQ, DIM)[None]).astype(np.float32)


# revision 65
# speedup vs baseline: 2.2882x; 2.2882x over previous
"""Trainium2 Bass kernel for nn_FLASH_40458591928592 (sparse_attention).

Sequence-sharded over 8 NeuronCores: 1024 tokens (= 4 groups of 256) per core.
Matmuls bf16 operands / fp32 PSUM, except:
  - v GEMM (x0 @ Wv): fp8e4m3 DoubleRow (2x PE rate), Wv host-scaled x64,
    silu applied with scale 1/64.
  - out GEMM (og @ W_out): first N8PAIRS*2 of 16 k-tiles in fp8 DoubleRow
    (og scaled x8 on chip, W_out x64 on host), rest bf16 with matching x512
    scale so all terms share one PSUM chain; evacuation scales by 1/512.
Residual x is added from bf16 (error contribution ~2e-3 rel).

Per-core device program:
  qk   = silu(x_s @ W_qk + b_qk)            feature-major  [128qk, 1024tok]
  v    = silu(x_0 @ W_h[:, :H] + b_h[:H])   token-major    [1024tok, 2048]
  gate = silu(x_s @ W_h[:, H:] + b_h[H:])   feature-major  [2048hc, 1024tok]
  Quadratic group attention with the causal depthwise conv folded in as a
  constant banded add to attnT plus a K=32 boundary matmul over the 32-token
  tail of the previous group (halo-computed for the first local group).
  lin_kv prefix scan: per-group kv accumulated in one PSUM bank with
  snapshots; per-core totals AllGathered per e-half (bf16); each core applies
  a 0/1 weighted sum (host-provided weights) for the exclusive prefix.
  out  = quadT + linT + convT               feature-major  [2048, 1024tok]
  y_s  = (gate_s*out)^T @ W_out + b_out + x_s  token-major, written fp32.

The first gate block (s0/th0) is emitted with deferred og-muls between
lin_half(0) and the eh1 collective consume, so the PE has ~27us of work
covering the second AllGather's latency.
"""

from contextlib import ExitStack

import numpy as np
import ml_dtypes

import concourse.tile as tile
from concourse import bacc, mybir
from concourse.bass_utils import run_bass_kernel_spmd
from concourse.masks import make_identity

BF = mybir.dt.bfloat16
F8 = mybir.dt.float8e4
F32 = mybir.dt.float32
bf16 = ml_dtypes.bfloat16
f8e4 = ml_dtypes.float8_e4m3

G = 256
DIM = 1024
HID = 2048
DQK = 128
NSEQ = 8192
NC = 8
T = NSEQ // NC        # 1024 tokens per core
NG = T // G           # 4 groups per core
KD = DIM // 128       # 8 k-tiles over dim
ET = HID // 128       # 16 e-tiles over hid
TT = T // 128         # 8 token tiles
EH = HID // 2         # 1024 cols per e-half

AF = mybir.ActivationFunctionType
ALU = mybir.AluOpType
DR = mybir.MatmulPerfMode.DoubleRow

WITH_VBIAS = True   # emit ones-row matmuls for b_h[:HID] (skipped when zero)
WITH_OBIAS = True   # emit ones-row matmuls for b_out (skipped when zero)

N8PAIRS = 6          # out-proj k-tile pairs in fp8 (of ET//2=8); rest bf16
S_W = 64.0           # host pre-scale on fp8 weights
S_OG = 8.0           # on-chip pre-scale on fp8/bf16 og
S_OUT = S_W * S_OG   # combined out-proj psum scale


def _emit(tc, ap):
    nc = tc.nc
    nf8 = N8PAIRS * 2          # fp8 k-tiles in out-proj
    with ExitStack() as ctx:
        consts = ctx.enter_context(tc.tile_pool(name="consts", bufs=1))
        p_xt = ctx.enter_context(tc.tile_pool(name="xt", bufs=3))
        p_xf8 = ctx.enter_context(tc.tile_pool(name="xf8", bufs=1))
        p_qkT = ctx.enter_context(tc.tile_pool(name="qkT", bufs=1))
        p_lk = ctx.enter_context(tc.tile_pool(name="lk", bufs=1))
        p_w = ctx.enter_context(tc.tile_pool(name="w", bufs=1))      # Wv -> Wg
        p_sw = ctx.enter_context(tc.tile_pool(name="sw", bufs=1))    # S_full -> Woutb
        p_w8 = ctx.enter_context(tc.tile_pool(name="w8", bufs=1))    # Wout8
        p_big = ctx.enter_context(tc.tile_pool(name="big", bufs=1))  # v_h -> gsilu
        p_og8 = ctx.enter_context(tc.tile_pool(name="og8", bufs=2))
        p_tails = ctx.enter_context(tc.tile_pool(name="tails", bufs=1))
        p_so = ctx.enter_context(tc.tile_pool(name="so", bufs=1))
        p_sob = ctx.enter_context(tc.tile_pool(name="sob", bufs=1))
        p_tr = ctx.enter_context(tc.tile_pool(name="tr", bufs=2))
        p_a0 = ctx.enter_context(tc.tile_pool(name="a0", bufs=4))
        p_a1 = ctx.enter_context(tc.tile_pool(name="a1", bufs=4))
        p_outT = ctx.enter_context(tc.tile_pool(name="outT", bufs=1))
        p_y = ctx.enter_context(tc.tile_pool(name="ysb", bufs=2))
        p_xr = ctx.enter_context(tc.tile_pool(name="xr", bufs=2))
        ps1 = ctx.enter_context(tc.tile_pool(name="ps1", bufs=4, space="PSUM"))
        ps2 = ctx.enter_context(tc.tile_pool(name="ps2", bufs=2, space="PSUM"))
        pskv = ctx.enter_context(tc.tile_pool(name="pskv", bufs=2, space="PSUM"))

        # warm-up collective FIRST: its ~50us setup barrier must finish
        # before the real AllGathers reach the cc stream (~70us in)
        warm = consts.tile([128, 16], BF, tag="warm")
        nc.vector.memset(warm, 0.0)
        nc.sync.dma_start(ap["cc_warm_in"], warm)
        nc.gpsimd.collective_compute(
            "AllGather", ALU.bypass, replica_groups=[list(range(NC))],
            ins=[ap["cc_warm_in"]], outs=[ap["cc_warm_out"]])

        # ---- the DMAs that gate the first PE work ----
        # consts/weights go on the ACT HWDGE queue so xt streams don't queue
        # behind them on SP
        wqk_sb = consts.tile([128, KD, DQK], BF, tag="wqk")
        nc.scalar.dma_start(wqk_sb, ap["wqk"].rearrange("(kt p) q -> p kt q", p=128))

        def load_xt(s):
            halves = []
            for h in range(2):
                t = p_xt.tile([128, KD // 2, T], BF, tag="xt")
                # one DMA per k-tile, halves split across the SP and GpSimd
                # HWDGE queues — one queue's engine fan-out (~100GB/s) is
                # less than the qk streams consume. GpSimd's h1 halves drain
                # late behind the warmup collective, but the extra aggregate
                # bandwidth still wins (measured: 624us vs 643us sync-only)
                eng = nc.sync if h == 0 else nc.gpsimd
                for q in range(4):
                    r0 = h * 512 + q * 128
                    eng.dma_start(
                        t[:, q:q + 1, :],
                        ap["xt"][s, r0:r0 + 128, :].rearrange(
                            "(kt p) t -> p kt t", p=128))
                halves.append(t)
            return halves

        def xt_k(halves, kt):
            return halves[kt // 4][:, kt % 4, :]

        bqk = consts.tile([128, 1], F32, tag="bqk")
        nc.scalar.dma_start(bqk, ap["bqk"])

        qkT = p_qkT.tile([128, 4, T], BF, tag="qkT")

        # prefetch Wv (fp8) early (first big PE phase after qk needs it)
        wv_sb = p_w.tile([128, KD, HID], F8, tag="wv8")
        nc.scalar.dma_start(wv_sb, ap["wv"].rearrange("(kt p) e -> p kt e", p=128))
        # x0 in fp8 for the v GEMM
        xf8 = p_xf8.tile([128, KD, T], F8, tag="xf8")
        nc.scalar.dma_start(xf8, ap["x0f8"].rearrange("(kt p) t -> p kt t", p=128))
        wout8_sb = p_w8.tile([128, max(nf8, 1), DIM], F8, tag="w8")

        def qk_stream(s, halves):
            # half-major: the stream starts once its first xt half lands;
            # ch0/ch1 share each LDWEIGHTS
            psa = ps1.tile([128, 512], F32, tag="ps1")
            psb = ps1.tile([128, 512], F32, tag="ps1")
            pss = [psa, psb]
            for h in range(2):
                for q in range(4):
                    kt = 4 * h + q
                    for ch in range(2):
                        nc.tensor.matmul(
                            pss[ch], wqk_sb[:, kt, :],
                            halves[h][:, q, ch * 512:(ch + 1) * 512],
                            start=(kt == 0), stop=(kt == KD - 1))
            for ch in range(2):
                nc.scalar.activation(qkT[:, s, ch * 512:(ch + 1) * 512], pss[ch],
                                     AF.Silu, bias=bqk, scale=1.0)

        halves3 = load_xt(3)

        # ---- constants (DMA on ACT queue while xt s3 streams) ----
        ident = consts.tile([128, 128], BF, tag="ident")
        make_identity(nc, ident)
        ones_t = None
        if WITH_VBIAS or WITH_OBIAS:
            ones_t = consts.tile([1, 128], BF, tag="ones")
            nc.vector.memset(ones_t, 1.0)
        triu = consts.tile([128, 128], BF, tag="triu")
        nc.scalar.dma_start(triu, ap["triu"])
        bdiag = consts.tile([128, 128], BF, tag="bdiag")
        nc.scalar.dma_start(bdiag, ap["bdiag"])
        bcorn = consts.tile([128, 128], BF, tag="bcorn")
        nc.scalar.dma_start(bcorn, ap["bcorn"])
        bprev = consts.tile([32, 32], BF, tag="bprev")
        nc.scalar.dma_start(bprev, ap["bprev"])
        hmask = consts.tile([32, 1], F32, tag="hmask")
        nc.scalar.dma_start(hmask, ap["hmask"])
        wsumw = consts.tile([128, NC], F32, tag="wsumw")
        nc.scalar.dma_start(wsumw, ap["wsumw"])
        bgate = consts.tile([128, ET], F32, tag="bgate")
        nc.scalar.dma_start(bgate, ap["bgate"])
        wvb = bout = None
        if WITH_VBIAS:
            wvb = consts.tile([1, HID], BF, tag="wvb")
            nc.scalar.dma_start(wvb, ap["wvb"])
        if WITH_OBIAS:
            bout = consts.tile([1, DIM], BF, tag="bout")
            nc.scalar.dma_start(bout, ap["bout"])
        xh_sb = consts.tile([128, KD, 32], F8, tag="xh")
        nc.scalar.dma_start(xh_sb, ap["xh"].rearrange("(kt p) t -> p kt t", p=128))

        qk_stream(3, halves3)

        # lk (stream 3) token-major via PE transpose
        lk_tok = p_lk.tile([128, TT, 128], BF, tag="lk")
        for tt in range(TT):
            pt = ps2.tile([128, 128], BF, tag="ps2")
            nc.tensor.transpose(pt, qkT[:, 3, tt * 128:(tt + 1) * 128], ident)
            nc.vector.tensor_copy(lk_tok[:, tt, :], pt)

        # ---- sim/attn per group (emitted after qk streams 0/2 are in) ----
        attn0, attn1 = [], []

        def attn_build():
            for g in range(NG):
                i0 = g * G
                a0 = p_a0.tile([128, 256], BF, tag="a0")
                ps = ps2.tile([128, 256], F32, tag="ps2")
                nc.tensor.matmul(ps, qkT[:, 2, i0:i0 + 128],
                                 qkT[:, 0, i0:i0 + 256], start=True, stop=True)
                nc.scalar.activation(a0, ps, AF.Relu, bias=0.0, scale=1.0 / G)
                nc.vector.tensor_mul(a0[:, 0:128], a0[:, 0:128], triu)
                nc.vector.tensor_mul(a0, a0, a0)
                nc.vector.tensor_add(a0[:, 0:128], a0[:, 0:128], bdiag)
                nc.vector.tensor_add(a0[:, 128:256], a0[:, 128:256], bcorn)
                attn0.append(a0)

                a1 = p_a1.tile([128, 128], BF, tag="a1")
                ps = ps2.tile([128, 256], F32, tag="ps2")
                nc.tensor.matmul(ps[:, 0:128], qkT[:, 2, i0 + 128:i0 + 256],
                                 qkT[:, 0, i0 + 128:i0 + 256],
                                 start=True, stop=True)
                nc.scalar.activation(a1, ps[:, 0:128], AF.Relu, bias=0.0,
                                     scale=1.0 / G)
                nc.vector.tensor_mul(a1, a1, triu)
                nc.vector.tensor_mul(a1, a1, a1)
                nc.vector.tensor_add(a1, a1, bdiag)
                attn1.append(a1)

        # ---- phases B-E per e-half ----
        S_full = p_sw.tile([128, NG, HID], BF, tag="sw")
        S_offb = p_sob.tile([128, HID], BF, tag="sob")

        def fire_half(eh, t_half):
            """DMA totals out and launch this half's AllGather (small payloads
            overlap compute far better than one combined collective)."""
            cc_in = ap[f"cc_in{eh}"]
            nc.sync.dma_start(cc_in, t_half)
            nc.gpsimd.collective_compute(
                "AllGather", ALU.bypass, replica_groups=[list(range(NC))],
                ins=[cc_in], outs=[ap[f"cc_out{eh}"]])

        def consume_half(eh):
            """Weighted-sum the gathered totals, add into S_full (DVE)."""
            e0 = eh * EH
            cc_out = ap[f"cc_out{eh}"]
            for r in range(NC):
                tr = p_tr.tile([128, EH], BF, tag="tr")
                nc.sync.dma_start(tr, cc_out[r * 128:(r + 1) * 128, :])
                if r == 0:
                    nc.vector.tensor_scalar_mul(S_offb[:, e0:e0 + EH], tr,
                                                wsumw[:, 0:1])
                else:
                    nc.vector.scalar_tensor_tensor(
                        S_offb[:, e0:e0 + EH], tr, wsumw[:, r:r + 1],
                        S_offb[:, e0:e0 + EH], op0=ALU.mult, op1=ALU.add)
            nc.vector.tensor_copy(S_full[:, 0, e0:e0 + EH], S_offb[:, e0:e0 + EH])
            for g in range(1, NG):
                nc.vector.tensor_add(S_full[:, g, e0:e0 + EH],
                                     S_full[:, g, e0:e0 + EH],
                                     S_offb[:, e0:e0 + EH])

        outT = p_outT.tile([128, ET, T], BF, tag="outT")

        def lin_half(eh):
            for g in range(NG):
                for et in range(eh * 8, eh * 8 + 8):
                    po = ps2.tile([128, 256], F32, tag="ps2")
                    nc.tensor.matmul(po, S_full[:, g, et * 128:(et + 1) * 128],
                                     qkT[:, 1, g * G:(g + 1) * G],
                                     start=True, stop=True)
                    nc.vector.tensor_add(outT[:, et, g * G:(g + 1) * G],
                                         outT[:, et, g * G:(g + 1) * G], po)

        def v_mm(ps, lhs_ap, e0, c0, w):
            """fp8 DoubleRow x@Wv chain into psum (scale S_W); w = token width."""
            for q in range(KD // 2):
                nc.tensor.matmul(
                    ps, lhs_ap[:, 2 * q:2 * q + 2, :],
                    wv_sb[:, 2 * q:2 * q + 2, e0 + c0:e0 + c0 + 512],
                    start=(q == 0),
                    stop=(q == KD // 2 - 1 and not WITH_VBIAS),
                    perf_mode=DR, skip_group_check=True)
            if WITH_VBIAS:
                nc.tensor.matmul(ps, ones_t[0:1, 0:w],
                                 wvb[0:1, e0 + c0:e0 + c0 + 512],
                                 start=False, stop=True, skip_group_check=True)

        def v_head(eh):
            v_h = p_big.tile([128, TT, EH], BF, tag="big")
            tails = p_tails.tile([32, NG, EH], BF, tag="tails")
            t_half = p_so.tile([128, EH], BF, tag="so")
            pk0 = pskv.tile([128, 512], F32, tag="pskv")
            pk1 = pskv.tile([128, 512], F32, tag="pskv")
            return dict(e0=eh * EH, v_h=v_h, tails=tails, t_half=t_half,
                        pk=[pk0, pk1])

        def v_pair(st, g):
            """Two v token-tiles + their kv-chain step (PSUM snapshot)."""
            e0, v_h, pk = st["e0"], st["v_h"], st["pk"]
            for tt in (2 * g, 2 * g + 1):
                for ec in range(2):
                    c0 = ec * 512
                    ps = ps1.tile([128, 512], F32, tag="ps1")
                    v_mm(ps, xf8[:, :, tt * 128:(tt + 1) * 128], e0, c0, 128)
                    nc.scalar.activation(v_h[:, tt, c0:c0 + 512], ps, AF.Silu,
                                         bias=0.0, scale=1.0 / S_W)
            for ec in range(2):
                c0 = ec * 512
                for jt in range(2):
                    nc.tensor.matmul(pk[ec], lk_tok[:, 2 * g + jt, :],
                                     v_h[:, 2 * g + jt, c0:c0 + 512],
                                     start=(g == 0 and jt == 0),
                                     stop=(g == NG - 1 and jt == 1),
                                     skip_group_check=True)
                dst = (S_full[:, g + 1, e0 + c0:e0 + c0 + 512]
                       if g < NG - 1 else st["t_half"][:, c0:c0 + 512])
                nc.scalar.activation(dst, pk[ec], AF.Copy, bias=0.0,
                                     scale=1.0 / G)

        def v_tail(eh, st):
            """Halo v for the conv boundary, group tails, fire the AllGather."""
            e0, v_h, tails = st["e0"], st["v_h"], st["tails"]
            for ec in range(2):
                c0 = ec * 512
                ps = ps1.tile([32, 512], F32, tag="ps1")
                v_mm(ps, xh_sb, e0, c0, 32)
                nc.scalar.activation(tails[:, 0, c0:c0 + 512], ps, AF.Silu,
                                     bias=0.0, scale=1.0 / S_W)
                nc.vector.tensor_scalar_mul(tails[:, 0, c0:c0 + 512],
                                            tails[:, 0, c0:c0 + 512], hmask)
            for g in range(1, NG):
                nc.sync.dma_start(tails[:, g, :], v_h[96:128, 2 * g - 1, :])
            fire_half(eh, st["t_half"])

        def quad_half(eh, st, g_order):
            v_h, tails = st["v_h"], st["tails"]
            for g in g_order:
                for et in range(8):
                    ec0 = et * 128
                    po = ps2.tile([128, 256], F32, tag="ps2")
                    nc.tensor.matmul(po, v_h[:, 2 * g, ec0:ec0 + 128], attn0[g],
                                     start=True, stop=False, skip_group_check=True)
                    nc.tensor.matmul(po[:, 128:256],
                                     v_h[:, 2 * g + 1, ec0:ec0 + 128],
                                     attn1[g], start=False, stop=False,
                                     skip_group_check=True)
                    nc.tensor.matmul(po[:, 0:32], tails[:, g, ec0:ec0 + 128], bprev,
                                     start=False, stop=True, skip_group_check=True)
                    nc.scalar.activation(outT[:, eh * 8 + et, g * G:(g + 1) * G],
                                         po, AF.Copy, bias=0.0, scale=1.0)

        # ---- interleaved: eh0 v-phase fills the qk streams' DMA windows ----
        st0 = v_head(0)
        v_pair(st0, 0)
        halves = load_xt(1)
        qk_stream(1, halves)
        v_pair(st0, 1)
        halves = load_xt(2)
        qk_stream(2, halves)
        v_pair(st0, 2)
        xt0 = load_xt(0)
        qk_stream(0, xt0)
        v_pair(st0, 3)
        # Wout fp8 part: first needed ~160us; on GpSimd AFTER the xt h1
        # loads so its 1.5MB stays out of the DMA-starved qk window
        if nf8:
            nc.gpsimd.dma_start(wout8_sb[:, 0:nf8, :],
                                ap["wout8"].rearrange("(kt p) n -> p kt n", p=128))
        v_tail(0, st0)          # fires AG0

        attn_build()
        consume_half(0)         # DVE parks on AG0 under quad0/eh1-v PE work
        quad_half(0, st0, range(NG))

        # ---- eh1 v-phase ----
        st1 = v_head(1)
        for g in range(NG):
            v_pair(st1, g)
        v_tail(1, st1)          # fires AG1; last Wv reader = the halo above

        # Wg chunked per-et so the gate can consume as it streams (the p_w
        # slot only frees after the eh1 halo); chunks alternate across two
        # queues — one queue's ~45GB/s starves the gate's 1.7us/et pace
        wg_sb = p_w.tile([128, KD, HID], BF, tag="w")
        for et in range(ET):
            eng = nc.scalar if et % 2 == 0 else nc.gpsimd
            eng.dma_start(
                wg_sb[:, :, et * 128:(et + 1) * 128],
                ap["wg"][:, et * 128:(et + 1) * 128].rearrange(
                    "(kt p) e -> p kt e", p=128))

        # Woutb (bf16 k-tiles) reuses the xf8 slot, dead after the eh1 halo;
        # loading here (not after lin1) keeps y_th0's bf16 chains fed
        woutb_sb = p_xf8.tile([128, max(ET - nf8, 1), DIM], BF, tag="xf8")
        for j in range(ET - nf8):
            nc.scalar.dma_start(woutb_sb[:, j, :],
                                ap["woutb"][j * 128:(j + 1) * 128, :])

        tc.no_sync_barrier()

        def gate_th(s, halves, th, defer_muls=False):
            t0 = th * 512
            gs = p_big.tile([128, ET, 512], BF, tag="big")
            for et in range(ET):
                ps = ps1.tile([128, 512], F32, tag="ps1")
                for kt in range(KD):
                    nc.tensor.matmul(
                        ps, wg_sb[:, kt, et * 128:(et + 1) * 128],
                        xt_k(halves, kt)[:, t0:t0 + 512],
                        start=(kt == 0), stop=(kt == KD - 1))
                nc.scalar.activation(gs[:, et, :], ps, AF.Silu,
                                     bias=bgate[:, et:et + 1], scale=1.0)
            if defer_muls:
                return gs, None
            return gs, og_muls(gs, th)

        def og_muls(gs, th, lo=0, hi=ET, og8=None):
            """og = S_OG * gate * outT; first nf8 k-tiles to fp8, rest bf16
            in place (so all out-proj terms carry the same x512 scale)."""
            t0 = th * 512
            if og8 is None and nf8:
                og8 = p_og8.tile([128, nf8, 512], F8, tag="og8")
            for et in range(lo, hi):
                if et < nf8:
                    nc.vector.scalar_tensor_tensor(
                        og8[:, et, :], gs[:, et, :], S_OG,
                        outT[:, et, t0:t0 + 512], op0=ALU.mult, op1=ALU.mult)
                else:
                    nc.vector.scalar_tensor_tensor(
                        gs[:, et, :], gs[:, et, :], S_OG,
                        outT[:, et, t0:t0 + 512], op0=ALU.mult, op1=ALU.mult)
            return og8

        def y_th(s, gs, og8, th):
            for tl in range(4):
                tt = th * 4 + tl
                for nch in range(2):
                    n0 = nch * 512
                    ps = ps1.tile([128, 512], F32, tag="ps1")
                    for p in range(N8PAIRS):
                        nc.tensor.matmul(
                            ps, og8[:, 2 * p:2 * p + 2, tl * 128:(tl + 1) * 128],
                            wout8_sb[:, 2 * p:2 * p + 2, n0:n0 + 512],
                            start=(p == 0),
                            stop=(nf8 == ET and p == N8PAIRS - 1
                                  and not WITH_OBIAS),
                            perf_mode=DR, skip_group_check=True)
                    for j, kt in enumerate(range(nf8, ET)):
                        nc.tensor.matmul(
                            ps, gs[:, kt, tl * 128:(tl + 1) * 128],
                            woutb_sb[:, j, n0:n0 + 512],
                            start=(N8PAIRS == 0 and j == 0),
                            stop=(kt == ET - 1 and not WITH_OBIAS),
                            skip_group_check=True)
                    if WITH_OBIAS:
                        nc.tensor.matmul(ps, ones_t[0:1, 0:128],
                                         bout[0:1, n0:n0 + 512],
                                         start=False, stop=True,
                                         skip_group_check=True)
                    xr = p_xr.tile([128, 512], BF, tag="xr")
                    nc.sync.dma_start(
                        xr, ap["xtok"][s, tt * 128:(tt + 1) * 128, n0:n0 + 512])
                    ysb = p_y.tile([128, 512], BF, tag="ysb")
                    nc.vector.scalar_tensor_tensor(
                        ysb, ps, 1.0 / S_OUT, xr, op0=ALU.mult, op1=ALU.add)
                    nc.sync.dma_start(
                        ap["y"][s, tt * 128:(tt + 1) * 128, n0:n0 + 512], ysb)

        # lin eh0 can run now (its collective was consumed during eh1's v);
        # then the deferred first gate block keeps the PE busy while the eh1
        # AllGather lands, after which its consume + lin eh1 complete outT.
        # quad1, lin0, then the deferred gate block cover AG1's latency on
        # the PE; quad1 must precede the gate block (its reads release the
        # p_big slot the gate's gs tile reuses). Group 0 last: its boundary
        # tails wait on the eh1 halo DVE mask.
        quad_half(1, st1, [1, 2, 3, 0])
        lin_half(0)
        gs0, _ = gate_th(0, xt0, 0, defer_muls=True)
        tc.no_sync_barrier()
        consume_half(1)
        lin_half(1)
        og8_0 = og_muls(gs0, 0)
        y_th(0, gs0, og8_0, 0)

        # ---- remaining gate + y blocks ----
        for s in range(4):
            halves = xt0 if s == 0 else load_xt(s)
            for th in range(2):
                if s == 0 and th == 0:
                    continue
                gs, og8 = gate_th(s, halves, th)
                y_th(s, gs, og8, th)


def build_nc(with_vbias=None, with_obias=None):
    global WITH_VBIAS, WITH_OBIAS
    if with_vbias is not None:
        WITH_VBIAS = with_vbias
    if with_obias is not None:
        WITH_OBIAS = with_obias
    nc = bacc.Bacc("TRN2", target_bir_lowering=False, debug=False, num_devices=NC)
    ap = {}
    nf8 = N8PAIRS * 2

    def dram(name, shape, dt, kind=None, addr_space=None):
        kw = {}
        if kind:
            kw["kind"] = kind
        if addr_space:
            kw["addr_space"] = addr_space
        ap[name] = nc.dram_tensor(name, shape, dt, **kw).ap()

    dram("xt", [4, DIM, T], BF, kind="ExternalInput")
    dram("x0f8", [DIM, T], F8, kind="ExternalInput")
    dram("xh", [DIM, 32], F8, kind="ExternalInput")
    dram("xtok", [4, T, DIM], BF, kind="ExternalInput")
    dram("wv", [DIM, HID], F8, kind="ExternalInput")
    dram("wg", [DIM, HID], BF, kind="ExternalInput")
    dram("wqk", [DIM, DQK], BF, kind="ExternalInput")
    dram("wout8", [max(nf8, 1) * 128, DIM], F8, kind="ExternalInput")
    dram("woutb", [max(ET - nf8, 1) * 128, DIM], BF, kind="ExternalInput")
    dram("wvb", [1, HID], BF, kind="ExternalInput")
    dram("bout", [1, DIM], BF, kind="ExternalInput")
    dram("bgate", [128, ET], F32, kind="ExternalInput")
    dram("bqk", [128, 1], F32, kind="ExternalInput")
    dram("triu", [128, 128], BF, kind="ExternalInput")
    dram("bdiag", [128, 128], BF, kind="ExternalInput")
    dram("bcorn", [128, 128], BF, kind="ExternalInput")
    dram("bprev", [32, 32], BF, kind="ExternalInput")
    dram("hmask", [32, 1], F32, kind="ExternalInput")
    dram("wsumw", [128, NC], F32, kind="ExternalInput")
    dram("cc_warm_in", [128, 16], BF)
    dram("cc_warm_out", [NC * 128, 16], BF, addr_space="Shared")
    dram("cc_in0", [128, EH], BF)
    dram("cc_out0", [NC * 128, EH], BF, addr_space="Shared")
    dram("cc_in1", [128, EH], BF)
    dram("cc_out1", [NC * 128, EH], BF, addr_space="Shared")
    dram("y", [4, T, DIM], BF, kind="ExternalOutput")

    with tile.TileContext(nc) as tc:
        _emit(tc, ap)
    nc.compile()
    return nc


def host_prep(inputs):
    """Pure layout transforms: shard, transpose, cast, build conv-band consts."""
    x = np.ascontiguousarray(np.asarray(inputs["x"], np.float32)[0])  # [4, N, DIM]
    W_h = np.asarray(inputs["W_h"], np.float32)
    b_h = np.asarray(inputs["b_h"], np.float32)
    W_qk = np.asarray(inputs["W_qk"], np.float32)
    b_qk = np.asarray(inputs["b_qk"], np.float32)
    W_out = np.asarray(inputs["W_out"], np.float32)
    b_out = np.asarray(inputs["b_out"], np.float32)
    cw = np.asarray(inputs["conv_w"], np.float32)
    nf8 = N8PAIRS * 2

    jj = np.arange(128)[:, None]
    ii = np.arange(128)[None, :]
    d = ii - jj
    triu = (ii >= jj).astype(bf16)
    bdiag = np.where((d >= 0) & (d <= 31), cw[np.clip(31 - d, 0, 62)], 0.0).astype(bf16)
    dc = (ii + 128) - jj
    bcorn = np.where((dc >= 0) & (dc <= 31),
                     cw[np.clip(31 - dc, 0, 62)], 0.0).astype(bf16)
    jt = np.arange(32)[:, None]
    ip = np.arange(32)[None, :]
    dp = ip + 32 - jt
    bprev = np.where((dp >= 1) & (dp <= 31),
                     cw[np.clip(31 - dp, 0, 62)], 0.0).astype(bf16)

    wout_s = W_out * S_W
    wout8 = wout_s[:nf8 * 128].astype(f8e4)
    if nf8 == 0:
        wout8 = np.zeros((128, DIM), f8e4)
    woutb = np.ascontiguousarray(wout_s[nf8 * 128:]).astype(bf16)
    if nf8 == ET:
        woutb = np.zeros((128, DIM), bf16)
    common = {
        "wv": (np.ascontiguousarray(W_h[:, :HID]) * S_W).astype(f8e4),
        "wg": np.ascontiguousarray(W_h[:, HID:]).astype(bf16),
        "wqk": W_qk.astype(bf16),
        "wout8": wout8,
        "woutb": woutb,
        "wvb": (b_h[None, :HID] * S_W).astype(bf16),
        "bout": (b_out[None, :] * S_OUT).astype(bf16),
        "bgate": np.ascontiguousarray(b_h[HID:].reshape(ET, 128).T).astype(np.float32),
        "bqk": b_qk[:, None].astype(np.float32),
        "triu": triu, "bdiag": bdiag, "bcorn": bcorn, "bprev": bprev,
    }

    in_maps = []
    for c in range(NC):
        sl = slice(c * T, (c + 1) * T)
        x_c = x[:, sl, :]
        xt = np.zeros((4, DIM, T), bf16)
        for s in range(4):
            xt[s] = x_c[s].T.astype(bf16)
        if c > 0:
            xh = np.ascontiguousarray(x[0, c * T - 32:c * T, :].T).astype(f8e4)
        else:
            xh = np.zeros((DIM, 32), f8e4)
        m = dict(common)
        m["xt"] = xt
        m["x0f8"] = np.ascontiguousarray(x_c[0].T).astype(f8e4)
        m["xh"] = xh
        m["xtok"] = np.ascontiguousarray(x_c).astype(bf16)
        m["hmask"] = np.full((32, 1), 1.0 if c > 0 else 0.0, np.float32)
        w = np.zeros((128, NC), np.float32)
        w[:, :c] = 1.0
        m["wsumw"] = w
        in_maps.append(m)
    return in_maps


_NC_PROG = None
_NC_FLAGS = None


def kernel(**inputs):
    global _NC_PROG, _NC_FLAGS
    b_h = np.asarray(inputs["b_h"], np.float32)
    b_out = np.asarray(inputs["b_out"], np.float32)
    flags = (bool(np.any(b_h[:HID])), bool(np.any(b_out)))
    if _NC_PROG is None or _NC_FLAGS != flags:
        _NC_PROG = build_nc(with_vbias=flags[0], with_obias=flags[1])
        _NC_FLAGS = flags
    in_maps = host_prep(inputs)
    res = run_bass_kernel_spmd(_NC_PROG, in_maps, list(range(NC)))
    y = np.stack([res.results[c]["y"] for c in range(NC)], axis=1)  # [4, NC, T, DIM]
    return np.ascontiguousarray(y.reshape(4, NSEQ, DIM)[None]).astype(np.float32)
